# revision 14
# baseline (speedup 1.0000x reference)
"""Trainium2 Bass kernel for EqualizedModConv2d — 1D Winograd F(3,3) along H.

out[3t+u, w'] = sum_q AT[u,q] * M[q][t, w'],   u in 0..2, t in 0..20
M[q][o,t,w']  = sum_{i,kw} WH[q,kw][o,i] * dh[q][i, t, w'+kw]    (PE, bf16)
WH[q,kw][o,i] = sum_kh G[q,kh] * w[o,i,kh,kw] * W_MUL            (host, bf16)
dh[q][i,t,w]  = BT[q] . xs[i, 3t:3t+5, w]                        (DVE, bf16)

AT = [[1,1,1,1,0],[0,1,-1,2,0],[0,1,1,4,1]]
BT = [[2,-1,-2,1,0],[0,-2,-1,1,0],[0,2,-3,1,0],[0,-1,0,1,0],[0,2,-1,-2,1]]
G  = [[1/2,0,0],[-1/2,-1/2,-1/2],[-1/6,1/6,-1/6],[1/6,1/3,2/3],[0,0,1]]

5 multiplies per 3 output rows vs 9 direct -> 1.8x fewer PE MACs.
21 tiles of 3 rows; tile 20 produces rows 60,61 (u=2 discarded); its input
row 64 is a zero pad. xs is staged per (sample, group-of-7-tiles) in slabs
of 8x3 rows (bf16, style-scaled by ACT from 24-row f32 DMA chunks).
Demod D=rsqrt(ssq@wsq+eps) from host-precomputed f32 wsq; output transform
on DVE (one ACT PSUM->SBUF copy of M1 per group to satisfy the one-PSUM-
operand-per-DVE-op rule), demod scaling + f32 conversion on ACT.
Distribution: data-parallel over batch, 2 samples per core on 8 cores.
"""

import sys
import types

import numpy as np

B, CIN, COUT, LATENT = 16, 512, 512, 512
H = W = 64
KH = KW = 3
OH = OW = 62
N_CORES = 8
BL = B // N_CORES
IC = CIN // 128
OC = COUT // 128
MUL_DENSE = float(LATENT ** -0.5)
W_MUL_CONV = float((CIN * KH * KW) ** -0.5)
EPS = 1e-8
NQ = 5            # Winograd points
NT = 7            # tiles per group
NG = 3            # groups (21 tiles total)

_cache = {}


def _ensure_ntff_hook():
    if "antenv.axon_hooks" in sys.modules:
        return
    try:
        import antenv
        from trn_agent_boot.trn_boot import _ntff_profile_via_ctypes
    except ImportError:
        return
    mod = types.ModuleType("antenv.axon_hooks")
    mod._hook = None

    def _set(h):
        mod._hook = h

    def _get():
        return mod._hook

    mod.set_axon_ntff_profile_hook = _set
    mod.get_axon_ntff_profile_hook = _get
    sys.modules["antenv.axon_hooks"] = mod
    antenv.axon_hooks = mod
    try:
        _set(_ntff_profile_via_ctypes("/opt/axon/libaxon_pjrt.so"))
    except OSError:
        pass


def build():
    import concourse.bass as bass
    import concourse.bacc as bacc
    import concourse.tile as tile
    from concourse import mybir

    f32 = mybir.dt.float32
    bf16 = mybir.dt.bfloat16
    AF = mybir.ActivationFunctionType
    ALU = mybir.AluOpType
    PSUM = bass.MemorySpace.PSUM

    nc = bacc.Bacc("TRN2", target_bir_lowering=False, debug=False)

    x_d = nc.dram_tensor("x", [BL, CIN, H, W], f32, kind="ExternalInput")
    # weights pre-split by cout block so the first conv block's weights land
    # first: [ot, i, tap, o_within]
    whT_d = nc.dram_tensor("whT", [OC, CIN, 15, 128], bf16, kind="ExternalInput")
    wsq_d = nc.dram_tensor("wsq", [CIN, COUT], bf16, kind="ExternalInput")
    dwt_d = nc.dram_tensor("dwt", [128, 4, CIN], bf16, kind="ExternalInput")
    yt_d = nc.dram_tensor("yt", [128, 4, BL], bf16, kind="ExternalInput")
    db_d = nc.dram_tensor("db", [128, 4, 1], f32, kind="ExternalInput")
    out_d = nc.dram_tensor("out", [BL, COUT, OH, OW], f32, kind="ExternalOutput")

    with tile.TileContext(nc) as tc:
        with (
            tc.tile_pool(name="persist", bufs=1) as persist,
            tc.tile_pool(name="xstage", bufs=4) as xstage,
            tc.tile_pool(name="slab", bufs=2) as slabp,
            tc.tile_pool(name="dhp", bufs=2) as dhp,
            tc.tile_pool(name="scr", bufs=6) as scr,
            tc.tile_pool(name="tp", bufs=8) as tp,
            tc.tile_pool(name="osb", bufs=3) as osb,
            tc.tile_pool(name="small", bufs=1) as small,
            tc.tile_pool(name="psc", bufs=8, space=PSUM) as psc,
        ):
            # conv weights [i, (ic, tap, o)], taps = q*3+kw, q in 0..4
            whb = persist.tile([128, OC, IC, 15, 128], bf16)
            wsq = persist.tile([128, IC, COUT], bf16)

            # ---- small param DMAs first ----
            dwt_sb = small.tile([128, 4, CIN], bf16)
            nc.sync.dma_start(dwt_sb[:, :, :], dwt_d[:, :, :])
            yt_sb = small.tile([128, 4, BL], bf16)
            nc.sync.dma_start(yt_sb[:, :, :], yt_d[:, :, :])
            db_sb = small.tile([128, 4, 1], f32)
            nc.sync.dma_start(db_sb[:, :, :], db_d[:, :, :])

            # ---- x slab staging: group g needs rows 21g .. 21g+22 (+2 pad
            #      rows 64/65 for g=2). DMA f32 chunks; ACT scales into bf16
            #      slab [128, IC, 8, 3, W] once s_sb exists. ----
            def dma_slab(b, g):
                tiles = []
                h0 = 21 * g
                rows = 23 if g < 2 else 22  # g2: rows 42..63
                for ic in range(IC):
                    xr = xstage.tile([128, 24, W], f32, tag="xr")
                    nc.sync.dma_start(
                        xr[:, 0:rows, :],
                        x_d[b, ic * 128:(ic + 1) * 128, h0:h0 + rows, :],
                    )
                    tiles.append(xr)
                return tiles

            def scale_slab(b, g, tiles, slab):
                # slab rows = global rows 21g .. 21g+23
                for ic in range(IC):
                    if g < 2:
                        # 23 real rows: 7 full hh groups + 2 rows
                        nc.scalar.activation(
                            slab[:, ic, 0:7, :, :],
                            tiles[ic][:, 0:21, :].rearrange("p (hh hp) w -> p hh hp w", hp=3),
                            AF.Copy, scale=s_sb[:, ic, b:b + 1],
                        )
                        nc.scalar.activation(
                            slab[:, ic, 7, 0:2, :], tiles[ic][:, 21:23, :],
                            AF.Copy, scale=s_sb[:, ic, b:b + 1],
                        )
                    else:
                        # 22 real rows (42..63) + zero rows 64,65
                        nc.scalar.activation(
                            slab[:, ic, 0:7, :, :],
                            tiles[ic][:, 0:21, :].rearrange("p (hh hp) w -> p hh hp w", hp=3),
                            AF.Copy, scale=s_sb[:, ic, b:b + 1],
                        )
                        nc.scalar.activation(
                            slab[:, ic, 7, 0, :], tiles[ic][:, 21, :],
                            AF.Copy, scale=s_sb[:, ic, b:b + 1],
                        )
                        nc.gpsimd.memset(slab[:, ic, 7, 1:3, :], 0.0)

            # ---- critical first slab split across BOTH HW-DGE rings; first
            #      cout-block's weights on the ACT ring right behind it ----
            def emit_whb(ot):
                for ic in range(IC):
                    i0 = ic * 128
                    nc.scalar.dma_start(
                        whb[:, ot, ic, :, :],
                        whT_d[ot, i0:i0 + 128, :, :],
                    )

            xt00 = []
            for ic in range(IC):
                xr = xstage.tile([128, 24, W], f32, tag="xr")
                eng = nc.sync if ic < 2 else nc.scalar
                eng.dma_start(xr[:, 0:23, :], x_d[0, ic * 128:(ic + 1) * 128, 0:23, :])
                xt00.append(xr)
            emit_whb(0)
            nc.sync.dma_start(wsq[:, :, :], wsq_d.ap().rearrange("(ic p) o -> p ic o", p=128))
            xt01 = dma_slab(0, 1)

            # ---- style: s = (y @ dense_w.T) * mul + b ; ssq = s^2 ----
            s_sb = small.tile([128, IC, BL], f32)
            ssq = small.tile([128, IC, BL], bf16)
            for ct in range(IC):
                ps = psc.tile([128, BL], f32, tag="convps")
                for lc in range(4):
                    nc.tensor.matmul(
                        ps[:, :],
                        dwt_sb[:, lc, ct * 128:(ct + 1) * 128],
                        yt_sb[:, lc, :],
                        start=(lc == 0),
                        stop=(lc == 3),
                    )
                nc.scalar.activation(
                    s_sb[:, ct, :], ps[:, :], AF.Identity,
                    bias=db_sb[:, ct, :], scale=MUL_DENSE,
                )
                nc.scalar.activation(ssq[:, ct, :], s_sb[:, ct, :], AF.Square)

            slab00 = slabp.tile([128, IC, 8, 3, W], bf16, tag="slab")
            for ic in range(IC):
                nc.vector.tensor_scalar_mul(
                    slab00[:, ic, 0:7, :, :],
                    xt00[ic][:, 0:21, :].rearrange("p (hh hp) w -> p hh hp w", hp=3),
                    s_sb[:, ic, 0:1],
                )
                nc.vector.tensor_scalar_mul(
                    slab00[:, ic, 7, 0:2, :], xt00[ic][:, 21:23, :],
                    s_sb[:, ic, 0:1],
                )

            # remaining cout blocks' weights (behind the slab00 scale in the
            # ACT queue; each lands just ahead of its conv block)
            emit_whb(1)
            emit_whb(2)
            emit_whb(3)

            # ---- demod ----
            sqrt_t = small.tile([128, OC, BL], f32)
            d_sb = small.tile([128, OC, BL], f32)
            eps_sb = small.tile([128, 1], f32)
            nc.gpsimd.memset(eps_sb[:, :], EPS)
            for ot in range(OC):
                ps = psc.tile([128, BL], f32, tag="convps")
                for ic in range(IC):
                    nc.tensor.matmul(
                        ps[:, :],
                        wsq[:, ic, ot * 128:(ot + 1) * 128],
                        ssq[:, ic, :],
                        start=(ic == 0),
                        stop=(ic == 3),
                    )
                nc.scalar.activation(
                    sqrt_t[:, ot, :], ps[:, :], AF.Sqrt,
                    bias=eps_sb[:, :], scale=1.0,
                )
                nc.vector.reciprocal(d_sb[:, ot, :], sqrt_t[:, ot, :])

            slab01 = slabp.tile([128, IC, 8, 3, W], bf16, tag="slab")
            scale_slab(0, 1, xt01, slab01)

            pending = {(0, 0): slab00, (0, 1): slab01}

            def get_slab(b, g):
                if (b, g) in pending:
                    return pending.pop((b, g))
                tiles = dma_slab(b, g)
                slab = slabp.tile([128, IC, 8, 3, W], bf16, tag="slab")
                scale_slab(b, g, tiles, slab)
                return slab

            # ---- main loop ----
            for b in range(BL):
                for g in range(NG):
                    slab = get_slab(b, g)
                    dh = dhp.tile([128, IC, NQ, NT, W], bf16, tag="dh")
                    for ic in range(IC):
                        # x_m = rows 3t+m for t in 0..6 (slab-local)
                        xm = [
                            slab[:, ic, 0:7, 0, :],
                            slab[:, ic, 0:7, 1, :],
                            slab[:, ic, 0:7, 2, :],
                            slab[:, ic, 1:8, 0, :],
                            slab[:, ic, 1:8, 1, :],
                        ]
                        e = scr.tile([128, NT, W], bf16, tag="scr")
                        t = scr.tile([128, NT, W], bf16, tag="scr")
                        t2 = scr.tile([128, NT, W], bf16, tag="scr")
                        gg = scr.tile([128, NT, W], bf16, tag="scr")
                        f = dh[:, ic, 3, :, :]  # q3 = x3 - x1, reused below
                        nc.vector.tensor_sub(e[:, :, :], xm[0], xm[2])
                        nc.vector.tensor_sub(f, xm[3], xm[1])
                        # q0 = 2e + f
                        nc.vector.scalar_tensor_tensor(
                            dh[:, ic, 0, :, :], e[:, :, :], 2.0, f, ALU.mult, ALU.add)
                        # q1 = -2x1 + (x3 - x2)
                        nc.vector.tensor_sub(t[:, :, :], xm[3], xm[2])
                        nc.vector.scalar_tensor_tensor(
                            dh[:, ic, 1, :, :], xm[1], -2.0, t[:, :, :], ALU.mult, ALU.add)
                        # q2 = 2x1 + (-3x2 + x3)
                        nc.vector.scalar_tensor_tensor(
                            t2[:, :, :], xm[2], -3.0, xm[3], ALU.mult, ALU.add)
                        nc.vector.scalar_tensor_tensor(
                            dh[:, ic, 2, :, :], xm[1], 2.0, t2[:, :, :], ALU.mult, ALU.add)
                        # q4 = -2f + (x4 - x2)
                        nc.vector.tensor_sub(gg[:, :, :], xm[4], xm[2])
                        nc.vector.scalar_tensor_tensor(
                            dh[:, ic, 4, :, :], f, -2.0, gg[:, :, :], ALU.mult, ALU.add)
                    for ot in range(OC):
                        o0 = ot * 128
                        ms = []
                        for q in range(NQ):
                            ps = psc.tile([128, NT, OW], f32, tag="convps")
                            for ic in range(IC):
                                for kw in range(KW):
                                    nc.tensor.matmul(
                                        ps[:, :, :],
                                        whb[:, ot, ic, q * 3 + kw, :],
                                        dh[:, ic, q, :, kw:kw + OW],
                                        start=(ic == 0 and kw == 0),
                                        stop=(ic == IC - 1 and kw == KW - 1),
                                    )
                            ms.append(ps)
                        # output transform: u0 = M0+M1+M2+M3, u1 = M1-M2+2M3,
                        # u2 = M1+M2+4M3+M4 (via c1 = ACT copy of M1)
                        c1 = tp.tile([128, NT, OW], f32, tag="t")
                        p = tp.tile([128, NT, OW], f32, tag="t")
                        m_ = tp.tile([128, NT, OW], f32, tag="t")
                        a = tp.tile([128, NT, OW], f32, tag="t")
                        u0 = tp.tile([128, NT, OW], f32, tag="t")
                        u1 = tp.tile([128, NT, OW], f32, tag="t")
                        b_ = tp.tile([128, NT, OW], f32, tag="t")
                        u2 = tp.tile([128, NT, OW], f32, tag="t")
                        nc.scalar.copy(c1[:, :, :], ms[1][:, :, :])
                        nc.vector.tensor_add(p[:, :, :], c1[:, :, :], ms[2][:, :, :])
                        nc.vector.tensor_sub(m_[:, :, :], c1[:, :, :], ms[2][:, :, :])
                        nc.vector.tensor_add(a[:, :, :], ms[0][:, :, :], p[:, :, :])
                        nc.vector.tensor_add(u0[:, :, :], a[:, :, :], ms[3][:, :, :])
                        nc.vector.scalar_tensor_tensor(
                            u1[:, :, :], ms[3][:, :, :], 2.0, m_[:, :, :], ALU.mult, ALU.add)
                        nc.vector.scalar_tensor_tensor(
                            b_[:, :, :], ms[3][:, :, :], 4.0, p[:, :, :], ALU.mult, ALU.add)
                        nc.vector.tensor_add(u2[:, :, :], b_[:, :, :], ms[4][:, :, :])
                        ob = osb.tile([128, NT, 3, OW], f32, tag="outsb")
                        dd = d_sb[:, ot, b:b + 1]
                        nc.scalar.activation(ob[:, :, 0, :], u0[:, :, :], AF.Copy, scale=dd)
                        nc.scalar.activation(ob[:, :, 1, :], u1[:, :, :], AF.Copy, scale=dd)
                        if g < 2:
                            nc.scalar.activation(ob[:, :, 2, :], u2[:, :, :], AF.Copy, scale=dd)
                            nc.sync.dma_start(
                                out_d[b, o0:o0 + 128, 21 * g:21 * g + 21, :],
                                ob[:, :, :, :],
                            )
                        else:
                            # rows 42..61: 6 full tiles + tile 20's u0,u1
                            nc.scalar.activation(ob[:, 0:6, 2, :], u2[:, 0:6, :], AF.Copy, scale=dd)
                            nc.sync.dma_start(
                                out_d[b, o0:o0 + 128, 42:60, :],
                                ob[:, 0:6, :, :],
                            )
                            nc.sync.dma_start(
                                out_d[b, o0:o0 + 128, 60:62, :],
                                ob[:, 6, 0:2, :],
                            )
                        # stage the next slab one group ahead, just after the
                        # first ot-block of each group (keeps ACT FIFO clear)
                        if ot == 0:
                            nxt = b * NG + g + 2  # global group index + 2
                            if nxt < BL * NG:
                                nb, ng = divmod(nxt, NG)
                                pending[(nb, ng)] = get_slab(nb, ng)

    nc.compile()
    return nc


def run(inputs, profile=False):
    import ml_dtypes
    from concourse.bass_utils import run_bass_kernel_spmd

    if "nc" not in _cache:
        _cache["nc"] = build()
    nc = _cache["nc"]

    x = np.ascontiguousarray(np.asarray(inputs["x"], dtype=np.float32))
    y = np.ascontiguousarray(np.asarray(inputs["y"], dtype=np.float32))
    dense_w = np.asarray(inputs["dense_w"], dtype=np.float32)
    dense_b = np.asarray(inputs["dense_b"], dtype=np.float32)
    weight = np.asarray(inputs["weight"], dtype=np.float32)

    G = np.array([[1 / 2, 0, 0],
                  [-1 / 2, -1 / 2, -1 / 2],
                  [-1 / 6, 1 / 6, -1 / 6],
                  [1 / 6, 1 / 3, 2 / 3],
                  [0, 0, 1]], np.float32)
    wm = weight * np.float32(W_MUL_CONV)
    wh = np.einsum('qk,oikw->iqwo', G, wm)                  # [i, 5, 3, o]
    whT = np.ascontiguousarray(
        wh.reshape(CIN, 15, OC, 128).transpose(2, 0, 1, 3)
        .astype(ml_dtypes.bfloat16))                        # [ot, i, tap, o]
    wsq_h = np.ascontiguousarray(
        np.sum(wm.astype(np.float64) ** 2, axis=(2, 3)).T
        .astype(ml_dtypes.bfloat16))
    dwt = np.ascontiguousarray(
        dense_w.T.reshape(4, 128, CIN).transpose(1, 0, 2)
        .astype(ml_dtypes.bfloat16))                        # [p, l-chunk, c]
    db = np.ascontiguousarray(
        dense_b.reshape(4, 128, 1).transpose(1, 0, 2))      # [p, c-chunk, 1]

    in_maps = []
    for c in range(N_CORES):
        sl = slice(c * BL, (c + 1) * BL)
        in_maps.append({
            "x": x[sl],
            "whT": whT,
            "wsq": wsq_h,
            "dwt": dwt,
            "yt": np.ascontiguousarray(y[sl].T.reshape(4, 128, BL).transpose(1, 0, 2).astype(ml_dtypes.bfloat16)),
            "db": db,
        })

    if profile:
        _ensure_ntff_hook()
    res = run_bass_kernel_spmd(
        nc, in_maps, core_ids=list(range(N_CORES)), trace=profile)
    out = np.concatenate([r["out"] for r in res.results], axis=0)
    return out, res.exec_time_ns


def kernel(**inputs) -> np.ndarray:
    out, _ = run(inputs)
    return out


# revision 15
# speedup vs baseline: 1.0078x; 1.0078x over previous
"""Trainium2 Bass kernel for EqualizedModConv2d — 1D Winograd F(3,3) along H.

out[3t+u, w'] = sum_q AT[u,q] * M[q][t, w'],   u in 0..2, t in 0..20
M[q][o,t,w']  = sum_{i,kw} WH[q,kw][o,i] * dh[q][i, t, w'+kw]    (PE, bf16)
WH[q,kw][o,i] = sum_kh G[q,kh] * w[o,i,kh,kw] * W_MUL            (host, bf16)
dh[q][i,t,w]  = BT[q] . xs[i, 3t:3t+5, w]                        (DVE, bf16)

AT = [[1,1,1,1,0],[0,1,-1,2,0],[0,1,1,4,1]]
BT = [[2,-1,-2,1,0],[0,-2,-1,1,0],[0,2,-3,1,0],[0,-1,0,1,0],[0,2,-1,-2,1]]
G  = [[1/2,0,0],[-1/2,-1/2,-1/2],[-1/6,1/6,-1/6],[1/6,1/3,2/3],[0,0,1]]

5 multiplies per 3 output rows vs 9 direct -> 1.8x fewer PE MACs.
21 tiles of 3 rows; tile 20 produces rows 60,61 (u=2 discarded); its input
row 64 is a zero pad. xs is staged per (sample, group-of-7-tiles) in slabs
of 8x3 rows (bf16, style-scaled by ACT from 24-row f32 DMA chunks).
Demod D=rsqrt(ssq@wsq+eps) from host-precomputed f32 wsq; output transform
on DVE (one ACT PSUM->SBUF copy of M1 per group to satisfy the one-PSUM-
operand-per-DVE-op rule), demod scaling + f32 conversion on ACT.
Distribution: data-parallel over batch, 2 samples per core on 8 cores.
"""

import sys
import types

import numpy as np

B, CIN, COUT, LATENT = 16, 512, 512, 512
H = W = 64
KH = KW = 3
OH = OW = 62
N_CORES = 8
BL = B // N_CORES
IC = CIN // 128
OC = COUT // 128
MUL_DENSE = float(LATENT ** -0.5)
W_MUL_CONV = float((CIN * KH * KW) ** -0.5)
EPS = 1e-8
NQ = 5            # Winograd points
NT = 7            # tiles per group
NG = 3            # groups (21 tiles total)

_cache = {}


def _ensure_ntff_hook():
    if "antenv.axon_hooks" in sys.modules:
        return
    try:
        import antenv
        from trn_agent_boot.trn_boot import _ntff_profile_via_ctypes
    except ImportError:
        return
    mod = types.ModuleType("antenv.axon_hooks")
    mod._hook = None

    def _set(h):
        mod._hook = h

    def _get():
        return mod._hook

    mod.set_axon_ntff_profile_hook = _set
    mod.get_axon_ntff_profile_hook = _get
    sys.modules["antenv.axon_hooks"] = mod
    antenv.axon_hooks = mod
    try:
        _set(_ntff_profile_via_ctypes("/opt/axon/libaxon_pjrt.so"))
    except OSError:
        pass


def build():
    import concourse.bass as bass
    import concourse.bacc as bacc
    import concourse.tile as tile
    from concourse import mybir

    f32 = mybir.dt.float32
    bf16 = mybir.dt.bfloat16
    AF = mybir.ActivationFunctionType
    ALU = mybir.AluOpType
    PSUM = bass.MemorySpace.PSUM

    nc = bacc.Bacc("TRN2", target_bir_lowering=False, debug=False)

    x_d = nc.dram_tensor("x", [BL, CIN, H, W], f32, kind="ExternalInput")
    # weights pre-split by cout block so the first conv block's weights land
    # first: [ot, i, tap, o_within]
    whT_d = nc.dram_tensor("whT", [OC, CIN, 15, 128], bf16, kind="ExternalInput")
    wsq_d = nc.dram_tensor("wsq", [CIN, COUT], bf16, kind="ExternalInput")
    dwt_d = nc.dram_tensor("dwt", [LATENT, CIN], bf16, kind="ExternalInput")
    yt_d = nc.dram_tensor("yt", [LATENT, BL], bf16, kind="ExternalInput")
    db_d = nc.dram_tensor("db", [CIN, 1], f32, kind="ExternalInput")
    out_d = nc.dram_tensor("out", [BL, COUT, OH, OW], f32, kind="ExternalOutput")

    with tile.TileContext(nc) as tc:
        with (
            tc.tile_pool(name="persist", bufs=1) as persist,
            tc.tile_pool(name="xstage", bufs=4) as xstage,
            tc.tile_pool(name="slab", bufs=2) as slabp,
            tc.tile_pool(name="dhp", bufs=2) as dhp,
            tc.tile_pool(name="scr", bufs=6) as scr,
            tc.tile_pool(name="tp", bufs=8) as tp,
            tc.tile_pool(name="osb", bufs=3) as osb,
            tc.tile_pool(name="small", bufs=1) as small,
            tc.tile_pool(name="psc", bufs=8, space=PSUM) as psc,
        ):
            # conv weights [i, (ic, tap, o)], taps = q*3+kw, q in 0..4
            whb = persist.tile([128, OC, IC, 15, 128], bf16)
            wsq = persist.tile([128, IC, COUT], bf16)

            # ---- small param DMAs first ----
            dwt_sb = small.tile([128, 4, CIN], bf16)
            nc.sync.dma_start(dwt_sb[:, :, :], dwt_d.ap().rearrange("(l p) c -> p l c", p=128))
            yt_sb = small.tile([128, 4, BL], bf16)
            nc.sync.dma_start(yt_sb[:, :, :], yt_d.ap().rearrange("(l p) b -> p l b", p=128))
            db_sb = small.tile([128, 4, 1], f32)
            nc.sync.dma_start(db_sb[:, :, :], db_d.ap().rearrange("(c p) u -> p c u", p=128))

            # ---- x slab staging: group g needs rows 21g .. 21g+22 (+2 pad
            #      rows 64/65 for g=2). DMA f32 chunks; ACT scales into bf16
            #      slab [128, IC, 8, 3, W] once s_sb exists. ----
            def dma_slab(b, g):
                tiles = []
                h0 = 21 * g
                rows = 23 if g < 2 else 22  # g2: rows 42..63
                for ic in range(IC):
                    xr = xstage.tile([128, 24, W], f32, tag="xr")
                    nc.sync.dma_start(
                        xr[:, 0:rows, :],
                        x_d[b, ic * 128:(ic + 1) * 128, h0:h0 + rows, :],
                    )
                    tiles.append(xr)
                return tiles

            def scale_slab(b, g, tiles, slab):
                # slab rows = global rows 21g .. 21g+23
                for ic in range(IC):
                    if g < 2:
                        # 23 real rows: 7 full hh groups + 2 rows
                        nc.scalar.activation(
                            slab[:, ic, 0:7, :, :],
                            tiles[ic][:, 0:21, :].rearrange("p (hh hp) w -> p hh hp w", hp=3),
                            AF.Copy, scale=s_sb[:, ic, b:b + 1],
                        )
                        nc.scalar.activation(
                            slab[:, ic, 7, 0:2, :], tiles[ic][:, 21:23, :],
                            AF.Copy, scale=s_sb[:, ic, b:b + 1],
                        )
                    else:
                        # 22 real rows (42..63) + zero rows 64,65
                        nc.scalar.activation(
                            slab[:, ic, 0:7, :, :],
                            tiles[ic][:, 0:21, :].rearrange("p (hh hp) w -> p hh hp w", hp=3),
                            AF.Copy, scale=s_sb[:, ic, b:b + 1],
                        )
                        nc.scalar.activation(
                            slab[:, ic, 7, 0, :], tiles[ic][:, 21, :],
                            AF.Copy, scale=s_sb[:, ic, b:b + 1],
                        )
                        nc.gpsimd.memset(slab[:, ic, 7, 1:3, :], 0.0)

            # ---- critical first slab split across BOTH HW-DGE rings; first
            #      cout-block's weights on the ACT ring right behind it ----
            def emit_whb(ot):
                for ic in range(IC):
                    i0 = ic * 128
                    nc.scalar.dma_start(
                        whb[:, ot, ic, :, :],
                        whT_d[ot, i0:i0 + 128, :, :],
                    )

            xt00 = []
            for ic in range(IC):
                xr = xstage.tile([128, 24, W], f32, tag="xr")
                eng = nc.sync if ic < 2 else nc.scalar
                eng.dma_start(xr[:, 0:23, :], x_d[0, ic * 128:(ic + 1) * 128, 0:23, :])
                xt00.append(xr)
            emit_whb(0)
            nc.sync.dma_start(wsq[:, :, :], wsq_d.ap().rearrange("(ic p) o -> p ic o", p=128))
            xt01 = dma_slab(0, 1)

            # ---- style: s = (y @ dense_w.T) * mul + b ; ssq = s^2 ----
            s_sb = small.tile([128, IC, BL], f32)
            ssq = small.tile([128, IC, BL], bf16)
            for ct in range(IC):
                ps = psc.tile([128, BL], f32, tag="convps")
                for lc in range(4):
                    nc.tensor.matmul(
                        ps[:, :],
                        dwt_sb[:, lc, ct * 128:(ct + 1) * 128],
                        yt_sb[:, lc, :],
                        start=(lc == 0),
                        stop=(lc == 3),
                    )
                nc.scalar.activation(
                    s_sb[:, ct, :], ps[:, :], AF.Identity,
                    bias=db_sb[:, ct, :], scale=MUL_DENSE,
                )
                nc.scalar.activation(ssq[:, ct, :], s_sb[:, ct, :], AF.Square)

            # PE keep-warm chain A (output never read; HAM stays at 8/8)
            psw = psc.tile([128, 512], f32, tag="convps")
            for i in range(24):
                nc.tensor.matmul(
                    psw[:, :], dwt_sb[:, 0, 0:128], dwt_sb[:, 1, :],
                    start=(i == 0), stop=(i == 23),
                )

            slab00 = slabp.tile([128, IC, 8, 3, W], bf16, tag="slab")
            scale_slab(0, 0, xt00, slab00)

            # remaining cout blocks' weights (behind the slab00 scale in the
            # ACT queue; each lands just ahead of its conv block)
            emit_whb(1)
            emit_whb(2)
            emit_whb(3)

            # ---- demod ----
            sqrt_t = small.tile([128, OC, BL], f32)
            d_sb = small.tile([128, OC, BL], f32)
            eps_sb = small.tile([128, 1], f32)
            nc.gpsimd.memset(eps_sb[:, :], EPS)
            for ot in range(OC):
                ps = psc.tile([128, BL], f32, tag="convps")
                for ic in range(IC):
                    nc.tensor.matmul(
                        ps[:, :],
                        wsq[:, ic, ot * 128:(ot + 1) * 128],
                        ssq[:, ic, :],
                        start=(ic == 0),
                        stop=(ic == 3),
                    )
                nc.scalar.activation(
                    sqrt_t[:, ot, :], ps[:, :], AF.Sqrt,
                    bias=eps_sb[:, :], scale=1.0,
                )
                nc.vector.reciprocal(d_sb[:, ot, :], sqrt_t[:, ot, :])

            # PE keep-warm chain B
            psw2 = psc.tile([128, 512], f32, tag="convps")
            for i in range(24):
                nc.tensor.matmul(
                    psw2[:, :], dwt_sb[:, 2, 0:128], dwt_sb[:, 3, :],
                    start=(i == 0), stop=(i == 23),
                )

            slab01 = slabp.tile([128, IC, 8, 3, W], bf16, tag="slab")
            scale_slab(0, 1, xt01, slab01)

            # bridge chain C: gated on slab00's scale, so it runs right
            # before the first conv block and hands it a warm PE
            psw3 = psc.tile([128, 448], f32, tag="convps")
            for i in range(12):
                nc.tensor.matmul(
                    psw3[:, :], dwt_sb[:, 0, 0:128],
                    slab00[:, 3, 0:7, 0, :],
                    start=(i == 0), stop=(i == 11),
                )

            pending = {(0, 0): slab00, (0, 1): slab01}

            def get_slab(b, g):
                if (b, g) in pending:
                    return pending.pop((b, g))
                tiles = dma_slab(b, g)
                slab = slabp.tile([128, IC, 8, 3, W], bf16, tag="slab")
                scale_slab(b, g, tiles, slab)
                return slab

            # ---- main loop ----
            for b in range(BL):
                for g in range(NG):
                    slab = get_slab(b, g)
                    dh = dhp.tile([128, IC, NQ, NT, W], bf16, tag="dh")
                    for ic in range(IC):
                        # x_m = rows 3t+m for t in 0..6 (slab-local)
                        xm = [
                            slab[:, ic, 0:7, 0, :],
                            slab[:, ic, 0:7, 1, :],
                            slab[:, ic, 0:7, 2, :],
                            slab[:, ic, 1:8, 0, :],
                            slab[:, ic, 1:8, 1, :],
                        ]
                        e = scr.tile([128, NT, W], bf16, tag="scr")
                        t = scr.tile([128, NT, W], bf16, tag="scr")
                        t2 = scr.tile([128, NT, W], bf16, tag="scr")
                        gg = scr.tile([128, NT, W], bf16, tag="scr")
                        f = dh[:, ic, 3, :, :]  # q3 = x3 - x1, reused below
                        nc.vector.tensor_sub(e[:, :, :], xm[0], xm[2])
                        nc.vector.tensor_sub(f, xm[3], xm[1])
                        # q0 = 2e + f
                        nc.vector.scalar_tensor_tensor(
                            dh[:, ic, 0, :, :], e[:, :, :], 2.0, f, ALU.mult, ALU.add)
                        # q1 = -2x1 + (x3 - x2)
                        nc.vector.tensor_sub(t[:, :, :], xm[3], xm[2])
                        nc.vector.scalar_tensor_tensor(
                            dh[:, ic, 1, :, :], xm[1], -2.0, t[:, :, :], ALU.mult, ALU.add)
                        # q2 = 2x1 + (-3x2 + x3)
                        nc.vector.scalar_tensor_tensor(
                            t2[:, :, :], xm[2], -3.0, xm[3], ALU.mult, ALU.add)
                        nc.vector.scalar_tensor_tensor(
                            dh[:, ic, 2, :, :], xm[1], 2.0, t2[:, :, :], ALU.mult, ALU.add)
                        # q4 = -2f + (x4 - x2)
                        nc.vector.tensor_sub(gg[:, :, :], xm[4], xm[2])
                        nc.vector.scalar_tensor_tensor(
                            dh[:, ic, 4, :, :], f, -2.0, gg[:, :, :], ALU.mult, ALU.add)
                    for ot in range(OC):
                        o0 = ot * 128
                        ms = []
                        for q in range(NQ):
                            ps = psc.tile([128, NT, OW], f32, tag="convps")
                            for ic in range(IC):
                                for kw in range(KW):
                                    nc.tensor.matmul(
                                        ps[:, :, :],
                                        whb[:, ot, ic, q * 3 + kw, :],
                                        dh[:, ic, q, :, kw:kw + OW],
                                        start=(ic == 0 and kw == 0),
                                        stop=(ic == IC - 1 and kw == KW - 1),
                                    )
                            ms.append(ps)
                        # output transform: u0 = M0+M1+M2+M3, u1 = M1-M2+2M3,
                        # u2 = M1+M2+4M3+M4 (via c1 = ACT copy of M1)
                        c1 = tp.tile([128, NT, OW], f32, tag="t")
                        p = tp.tile([128, NT, OW], f32, tag="t")
                        m_ = tp.tile([128, NT, OW], f32, tag="t")
                        a = tp.tile([128, NT, OW], f32, tag="t")
                        u0 = tp.tile([128, NT, OW], f32, tag="t")
                        u1 = tp.tile([128, NT, OW], f32, tag="t")
                        b_ = tp.tile([128, NT, OW], f32, tag="t")
                        u2 = tp.tile([128, NT, OW], f32, tag="t")
                        nc.scalar.copy(c1[:, :, :], ms[1][:, :, :])
                        nc.vector.tensor_add(p[:, :, :], c1[:, :, :], ms[2][:, :, :])
                        nc.vector.tensor_sub(m_[:, :, :], c1[:, :, :], ms[2][:, :, :])
                        nc.vector.tensor_add(a[:, :, :], ms[0][:, :, :], p[:, :, :])
                        nc.vector.tensor_add(u0[:, :, :], a[:, :, :], ms[3][:, :, :])
                        nc.vector.scalar_tensor_tensor(
                            u1[:, :, :], ms[3][:, :, :], 2.0, m_[:, :, :], ALU.mult, ALU.add)
                        nc.vector.scalar_tensor_tensor(
                            b_[:, :, :], ms[3][:, :, :], 4.0, p[:, :, :], ALU.mult, ALU.add)
                        nc.vector.tensor_add(u2[:, :, :], b_[:, :, :], ms[4][:, :, :])
                        ob = osb.tile([128, NT, 3, OW], f32, tag="outsb")
                        dd = d_sb[:, ot, b:b + 1]
                        nc.scalar.activation(ob[:, :, 0, :], u0[:, :, :], AF.Copy, scale=dd)
                        nc.scalar.activation(ob[:, :, 1, :], u1[:, :, :], AF.Copy, scale=dd)
                        if g < 2:
                            nc.scalar.activation(ob[:, :, 2, :], u2[:, :, :], AF.Copy, scale=dd)
                            nc.sync.dma_start(
                                out_d[b, o0:o0 + 128, 21 * g:21 * g + 21, :],
                                ob[:, :, :, :],
                            )
                        else:
                            # rows 42..61: 6 full tiles + tile 20's u0,u1
                            nc.scalar.activation(ob[:, 0:6, 2, :], u2[:, 0:6, :], AF.Copy, scale=dd)
                            nc.sync.dma_start(
                                out_d[b, o0:o0 + 128, 42:60, :],
                                ob[:, 0:6, :, :],
                            )
                            nc.sync.dma_start(
                                out_d[b, o0:o0 + 128, 60:62, :],
                                ob[:, 6, 0:2, :],
                            )
                        # stage the next slab one group ahead, just after the
                        # first ot-block of each group (keeps ACT FIFO clear)
                        if ot == 0:
                            nxt = b * NG + g + 2  # global group index + 2
                            if nxt < BL * NG:
                                nb, ng = divmod(nxt, NG)
                                pending[(nb, ng)] = get_slab(nb, ng)

    nc.compile()
    return nc


def run(inputs, profile=False):
    import ml_dtypes
    from concourse.bass_utils import run_bass_kernel_spmd

    if "nc" not in _cache:
        _cache["nc"] = build()
    nc = _cache["nc"]

    x = np.ascontiguousarray(np.asarray(inputs["x"], dtype=np.float32))
    y = np.ascontiguousarray(np.asarray(inputs["y"], dtype=np.float32))
    dense_w = np.asarray(inputs["dense_w"], dtype=np.float32)
    dense_b = np.asarray(inputs["dense_b"], dtype=np.float32)
    weight = np.asarray(inputs["weight"], dtype=np.float32)

    G = np.array([[1 / 2, 0, 0],
                  [-1 / 2, -1 / 2, -1 / 2],
                  [-1 / 6, 1 / 6, -1 / 6],
                  [1 / 6, 1 / 3, 2 / 3],
                  [0, 0, 1]], np.float32)
    wm = weight * np.float32(W_MUL_CONV)
    wh = np.einsum('qk,oikw->iqwo', G, wm)                  # [i, 5, 3, o]
    whT = np.ascontiguousarray(
        wh.reshape(CIN, 15, OC, 128).transpose(2, 0, 1, 3)
        .astype(ml_dtypes.bfloat16))                        # [ot, i, tap, o]
    wsq_h = np.ascontiguousarray(
        np.sum(wm.astype(np.float64) ** 2, axis=(2, 3)).T
        .astype(ml_dtypes.bfloat16))
    dwt = np.ascontiguousarray(dense_w.T.astype(ml_dtypes.bfloat16))
    db = np.ascontiguousarray(dense_b.reshape(CIN, 1))

    in_maps = []
    for c in range(N_CORES):
        sl = slice(c * BL, (c + 1) * BL)
        in_maps.append({
            "x": x[sl],
            "whT": whT,
            "wsq": wsq_h,
            "dwt": dwt,
            "yt": np.ascontiguousarray(y[sl].T.astype(ml_dtypes.bfloat16)),
            "db": db,
        })

    if profile:
        _ensure_ntff_hook()
    res = run_bass_kernel_spmd(
        nc, in_maps, core_ids=list(range(N_CORES)), trace=profile)
    out = np.concatenate([r["out"] for r in res.results], axis=0)
    return out, res.exec_time_ns


def kernel(**inputs) -> np.ndarray:
    out, _ = run(inputs)
    return out


# revision 17
# speedup vs baseline: 1.0196x; 1.0117x over previous
"""Trainium2 Bass kernel for EqualizedModConv2d — 1D Winograd F(3,3) along H.

out[3t+u, w'] = sum_q AT[u,q] * M[q][t, w'],   u in 0..2, t in 0..20
M[q][o,t,w']  = sum_{i,kw} WH[q,kw][o,i] * dh[q][i, t, w'+kw]    (PE, bf16)
WH[q,kw][o,i] = sum_kh G[q,kh] * w[o,i,kh,kw] * W_MUL            (host, bf16)
dh[q][i,t,w]  = BT[q] . xs[i, 3t:3t+5, w]                        (DVE, bf16)

AT = [[1,1,1,1,0],[0,1,-1,2,0],[0,1,1,4,1]]
BT = [[2,-1,-2,1,0],[0,-2,-1,1,0],[0,2,-3,1,0],[0,-1,0,1,0],[0,2,-1,-2,1]]
G  = [[1/2,0,0],[-1/2,-1/2,-1/2],[-1/6,1/6,-1/6],[1/6,1/3,2/3],[0,0,1]]

5 multiplies per 3 output rows vs 9 direct -> 1.8x fewer PE MACs.
21 tiles of 3 rows; tile 20 produces rows 60,61 (u=2 discarded); its input
row 64 is a zero pad. xs is staged per (sample, group-of-7-tiles) in slabs
of 8x3 rows (bf16, style-scaled by ACT from 24-row f32 DMA chunks).
Demod D=rsqrt(ssq@wsq+eps) from host-precomputed f32 wsq; output transform
on DVE (one ACT PSUM->SBUF copy of M1 per group to satisfy the one-PSUM-
operand-per-DVE-op rule), demod scaling + f32 conversion on ACT.
Distribution: data-parallel over batch, 2 samples per core on 8 cores.
"""

import sys
import types

import numpy as np

B, CIN, COUT, LATENT = 16, 512, 512, 512
H = W = 64
KH = KW = 3
OH = OW = 62
N_CORES = 8
BL = B // N_CORES
IC = CIN // 128
OC = COUT // 128
MUL_DENSE = float(LATENT ** -0.5)
W_MUL_CONV = float((CIN * KH * KW) ** -0.5)
EPS = 1e-8
NQ = 5            # Winograd points
NT = 7            # tiles per group
NG = 3            # groups (21 tiles total)

_cache = {}


def _ensure_ntff_hook():
    if "antenv.axon_hooks" in sys.modules:
        return
    try:
        import antenv
        from trn_agent_boot.trn_boot import _ntff_profile_via_ctypes
    except ImportError:
        return
    mod = types.ModuleType("antenv.axon_hooks")
    mod._hook = None

    def _set(h):
        mod._hook = h

    def _get():
        return mod._hook

    mod.set_axon_ntff_profile_hook = _set
    mod.get_axon_ntff_profile_hook = _get
    sys.modules["antenv.axon_hooks"] = mod
    antenv.axon_hooks = mod
    try:
        _set(_ntff_profile_via_ctypes("/opt/axon/libaxon_pjrt.so"))
    except OSError:
        pass


def build():
    import concourse.bass as bass
    import concourse.bacc as bacc
    import concourse.tile as tile
    from concourse import mybir

    f32 = mybir.dt.float32
    bf16 = mybir.dt.bfloat16
    AF = mybir.ActivationFunctionType
    ALU = mybir.AluOpType
    PSUM = bass.MemorySpace.PSUM

    nc = bacc.Bacc("TRN2", target_bir_lowering=False, debug=False)

    x_d = nc.dram_tensor("x", [BL, CIN, H, W], f32, kind="ExternalInput")
    # weights pre-split by cout block so the first conv block's weights land
    # first: [ot, i, tap, o_within]
    whT_d = nc.dram_tensor("whT", [OC, CIN, 15, 128], bf16, kind="ExternalInput")
    wsq_d = nc.dram_tensor("wsq", [CIN, COUT], bf16, kind="ExternalInput")
    dwt_d = nc.dram_tensor("dwt", [128, 4, CIN], bf16, kind="ExternalInput")
    yt_d = nc.dram_tensor("yt", [128, 4, BL], bf16, kind="ExternalInput")
    db_d = nc.dram_tensor("db", [128, 4, 1], f32, kind="ExternalInput")
    out_d = nc.dram_tensor("out", [BL, COUT, OH, OW], f32, kind="ExternalOutput")

    with tile.TileContext(nc) as tc:
        with (
            tc.tile_pool(name="persist", bufs=1) as persist,
            tc.tile_pool(name="xstage", bufs=4) as xstage,
            tc.tile_pool(name="slab", bufs=2) as slabp,
            tc.tile_pool(name="dhp", bufs=2) as dhp,
            tc.tile_pool(name="scr", bufs=6) as scr,
            tc.tile_pool(name="tp", bufs=8) as tp,
            tc.tile_pool(name="osb", bufs=3) as osb,
            tc.tile_pool(name="small", bufs=1) as small,
            tc.tile_pool(name="psc", bufs=8, space=PSUM) as psc,
        ):
            # conv weights [i, (ic, tap, o)], taps = q*3+kw, q in 0..4
            whb = persist.tile([128, OC, IC, 15, 128], bf16)
            wsq = persist.tile([128, IC, COUT], bf16)

            # ---- small param DMAs first ----
            dwt_sb = small.tile([128, 4, CIN], bf16)
            nc.sync.dma_start(dwt_sb[:, :, :], dwt_d[:, :, :])
            yt_sb = small.tile([128, 4, BL], bf16)
            nc.sync.dma_start(yt_sb[:, :, :], yt_d[:, :, :])
            db_sb = small.tile([128, 4, 1], f32)
            nc.sync.dma_start(db_sb[:, :, :], db_d[:, :, :])

            # ---- x slab staging: group g needs rows 21g .. 21g+22 (+2 pad
            #      rows 64/65 for g=2). DMA f32 chunks; ACT scales into bf16
            #      slab [128, IC, 8, 3, W] once s_sb exists. ----
            def dma_slab(b, g):
                tiles = []
                h0 = 21 * g
                rows = 23 if g < 2 else 22  # g2: rows 42..63
                for ic in range(IC):
                    xr = xstage.tile([128, 24, W], f32, tag="xr")
                    nc.sync.dma_start(
                        xr[:, 0:rows, :],
                        x_d[b, ic * 128:(ic + 1) * 128, h0:h0 + rows, :],
                    )
                    tiles.append(xr)
                return tiles

            def scale_slab(b, g, tiles, slab):
                # slab rows = global rows 21g .. 21g+23
                for ic in range(IC):
                    if g < 2:
                        # 23 real rows: 7 full hh groups + 2 rows
                        nc.scalar.activation(
                            slab[:, ic, 0:7, :, :],
                            tiles[ic][:, 0:21, :].rearrange("p (hh hp) w -> p hh hp w", hp=3),
                            AF.Copy, scale=s_sb[:, ic, b:b + 1],
                        )
                        nc.scalar.activation(
                            slab[:, ic, 7, 0:2, :], tiles[ic][:, 21:23, :],
                            AF.Copy, scale=s_sb[:, ic, b:b + 1],
                        )
                    else:
                        # 22 real rows (42..63) + zero rows 64,65
                        nc.scalar.activation(
                            slab[:, ic, 0:7, :, :],
                            tiles[ic][:, 0:21, :].rearrange("p (hh hp) w -> p hh hp w", hp=3),
                            AF.Copy, scale=s_sb[:, ic, b:b + 1],
                        )
                        nc.scalar.activation(
                            slab[:, ic, 7, 0, :], tiles[ic][:, 21, :],
                            AF.Copy, scale=s_sb[:, ic, b:b + 1],
                        )
                        nc.gpsimd.memset(slab[:, ic, 7, 1:3, :], 0.0)

            # ---- critical first slab split across BOTH HW-DGE rings; first
            #      cout-block's weights on the ACT ring right behind it ----
            def emit_whb(ot):
                for ic in range(IC):
                    i0 = ic * 128
                    nc.scalar.dma_start(
                        whb[:, ot, ic, :, :],
                        whT_d[ot, i0:i0 + 128, :, :],
                    )

            xt00 = []
            for ic in range(IC):
                xr = xstage.tile([128, 24, W], f32, tag="xr")
                eng = nc.sync if ic < 2 else nc.scalar
                eng.dma_start(xr[:, 0:23, :], x_d[0, ic * 128:(ic + 1) * 128, 0:23, :])
                xt00.append(xr)
            emit_whb(0)
            nc.sync.dma_start(wsq[:, :, :], wsq_d.ap().rearrange("(ic p) o -> p ic o", p=128))
            xt01 = dma_slab(0, 1)

            # ---- style: s = (y @ dense_w.T) * mul + b ; ssq = s^2 ----
            s_sb = small.tile([128, IC, BL], f32)
            ssq = small.tile([128, IC, BL], bf16)
            for ct in range(IC):
                ps = psc.tile([128, BL], f32, tag="convps")
                for lc in range(4):
                    nc.tensor.matmul(
                        ps[:, :],
                        dwt_sb[:, lc, ct * 128:(ct + 1) * 128],
                        yt_sb[:, lc, :],
                        start=(lc == 0),
                        stop=(lc == 3),
                    )
                nc.scalar.activation(
                    s_sb[:, ct, :], ps[:, :], AF.Identity,
                    bias=db_sb[:, ct, :], scale=MUL_DENSE,
                )
                nc.scalar.activation(ssq[:, ct, :], s_sb[:, ct, :], AF.Square)

            slab00 = slabp.tile([128, IC, 8, 3, W], bf16, tag="slab")
            scale_slab(0, 0, xt00, slab00)

            # remaining cout blocks' weights (behind the slab00 scale in the
            # ACT queue; each lands just ahead of its conv block)
            emit_whb(1)
            emit_whb(2)
            emit_whb(3)

            # ---- demod ----
            sqrt_t = small.tile([128, OC, BL], f32)
            d_sb = small.tile([128, OC, BL], f32)
            eps_sb = small.tile([128, 1], f32)
            nc.gpsimd.memset(eps_sb[:, :], EPS)
            for ot in range(OC):
                ps = psc.tile([128, BL], f32, tag="convps")
                for ic in range(IC):
                    nc.tensor.matmul(
                        ps[:, :],
                        wsq[:, ic, ot * 128:(ot + 1) * 128],
                        ssq[:, ic, :],
                        start=(ic == 0),
                        stop=(ic == 3),
                    )
                nc.scalar.activation(
                    sqrt_t[:, ot, :], ps[:, :], AF.Sqrt,
                    bias=eps_sb[:, :], scale=1.0,
                )
                nc.vector.reciprocal(d_sb[:, ot, :], sqrt_t[:, ot, :])

            slab01 = slabp.tile([128, IC, 8, 3, W], bf16, tag="slab")
            scale_slab(0, 1, xt01, slab01)

            pending = {(0, 0): slab00, (0, 1): slab01}

            def get_slab(b, g):
                if (b, g) in pending:
                    return pending.pop((b, g))
                tiles = dma_slab(b, g)
                slab = slabp.tile([128, IC, 8, 3, W], bf16, tag="slab")
                scale_slab(b, g, tiles, slab)
                return slab

            # ---- main loop ----
            for b in range(BL):
                for g in range(NG):
                    slab = get_slab(b, g)
                    dh = dhp.tile([128, IC, NQ, NT, W], bf16, tag="dh")

                    def xmv(ic):
                        return [
                            slab[:, ic, 0:7, 0, :],
                            slab[:, ic, 0:7, 1, :],
                            slab[:, ic, 0:7, 2, :],
                            slab[:, ic, 1:8, 0, :],
                            slab[:, ic, 1:8, 1, :],
                        ]

                    # phase 1: q0 + q3 for every cin chunk (unblocks the
                    # first conv accumulation chain early)
                    for ic in range(IC):
                        xm = xmv(ic)
                        e = scr.tile([128, NT, W], bf16, tag="scr")
                        f = dh[:, ic, 3, :, :]  # q3 = x3 - x1, reused below
                        nc.vector.tensor_sub(e[:, :, :], xm[0], xm[2])
                        nc.vector.tensor_sub(f, xm[3], xm[1])
                        # q0 = 2e + f
                        nc.vector.scalar_tensor_tensor(
                            dh[:, ic, 0, :, :], e[:, :, :], 2.0, f, ALU.mult, ALU.add)
                    # phase 2: q1, q2, q4
                    for ic in range(IC):
                        xm = xmv(ic)
                        f = dh[:, ic, 3, :, :]
                        t = scr.tile([128, NT, W], bf16, tag="scr")
                        t2 = scr.tile([128, NT, W], bf16, tag="scr")
                        gg = scr.tile([128, NT, W], bf16, tag="scr")
                        # q1 = -2x1 + (x3 - x2)
                        nc.vector.tensor_sub(t[:, :, :], xm[3], xm[2])
                        nc.vector.scalar_tensor_tensor(
                            dh[:, ic, 1, :, :], xm[1], -2.0, t[:, :, :], ALU.mult, ALU.add)
                        # q2 = 2x1 + (-3x2 + x3)
                        nc.vector.scalar_tensor_tensor(
                            t2[:, :, :], xm[2], -3.0, xm[3], ALU.mult, ALU.add)
                        nc.vector.scalar_tensor_tensor(
                            dh[:, ic, 2, :, :], xm[1], 2.0, t2[:, :, :], ALU.mult, ALU.add)
                        # q4 = -2f + (x4 - x2)
                        nc.vector.tensor_sub(gg[:, :, :], xm[4], xm[2])
                        nc.vector.scalar_tensor_tensor(
                            dh[:, ic, 4, :, :], f, -2.0, gg[:, :, :], ALU.mult, ALU.add)
                    for ot in range(OC):
                        o0 = ot * 128
                        ms = []
                        for q in range(NQ):
                            ps = psc.tile([128, NT, OW], f32, tag="convps")
                            for ic in range(IC):
                                for kw in range(KW):
                                    nc.tensor.matmul(
                                        ps[:, :, :],
                                        whb[:, ot, ic, q * 3 + kw, :],
                                        dh[:, ic, q, :, kw:kw + OW],
                                        start=(ic == 0 and kw == 0),
                                        stop=(ic == IC - 1 and kw == KW - 1),
                                    )
                            ms.append(ps)
                        # output transform: u0 = M0+M1+M2+M3, u1 = M1-M2+2M3,
                        # u2 = M1+M2+4M3+M4 (via c1 = ACT copy of M1)
                        c1 = tp.tile([128, NT, OW], f32, tag="t")
                        p = tp.tile([128, NT, OW], f32, tag="t")
                        m_ = tp.tile([128, NT, OW], f32, tag="t")
                        a = tp.tile([128, NT, OW], f32, tag="t")
                        u0 = tp.tile([128, NT, OW], f32, tag="t")
                        u1 = tp.tile([128, NT, OW], f32, tag="t")
                        b_ = tp.tile([128, NT, OW], f32, tag="t")
                        u2 = tp.tile([128, NT, OW], f32, tag="t")
                        nc.scalar.copy(c1[:, :, :], ms[1][:, :, :])
                        nc.vector.tensor_add(p[:, :, :], c1[:, :, :], ms[2][:, :, :])
                        nc.vector.tensor_sub(m_[:, :, :], c1[:, :, :], ms[2][:, :, :])
                        nc.vector.tensor_add(a[:, :, :], ms[0][:, :, :], p[:, :, :])
                        nc.vector.tensor_add(u0[:, :, :], a[:, :, :], ms[3][:, :, :])
                        nc.vector.scalar_tensor_tensor(
                            u1[:, :, :], ms[3][:, :, :], 2.0, m_[:, :, :], ALU.mult, ALU.add)
                        nc.vector.scalar_tensor_tensor(
                            b_[:, :, :], ms[3][:, :, :], 4.0, p[:, :, :], ALU.mult, ALU.add)
                        nc.vector.tensor_add(u2[:, :, :], b_[:, :, :], ms[4][:, :, :])
                        ob = osb.tile([128, NT, 3, OW], f32, tag="outsb")
                        dd = d_sb[:, ot, b:b + 1]
                        nc.scalar.activation(ob[:, :, 0, :], u0[:, :, :], AF.Copy, scale=dd)
                        nc.scalar.activation(ob[:, :, 1, :], u1[:, :, :], AF.Copy, scale=dd)
                        if g < 2:
                            nc.scalar.activation(ob[:, :, 2, :], u2[:, :, :], AF.Copy, scale=dd)
                            nc.sync.dma_start(
                                out_d[b, o0:o0 + 128, 21 * g:21 * g + 21, :],
                                ob[:, :, :, :],
                            )
                        else:
                            # rows 42..61: 6 full tiles + tile 20's u0,u1
                            nc.scalar.activation(ob[:, 0:6, 2, :], u2[:, 0:6, :], AF.Copy, scale=dd)
                            nc.sync.dma_start(
                                out_d[b, o0:o0 + 128, 42:60, :],
                                ob[:, 0:6, :, :],
                            )
                            nc.sync.dma_start(
                                out_d[b, o0:o0 + 128, 60:62, :],
                                ob[:, 6, 0:2, :],
                            )
                        # stage the next slab one group ahead, just after the
                        # first ot-block of each group (keeps ACT FIFO clear)
                        if ot == 0:
                            nxt = b * NG + g + 2  # global group index + 2
                            if nxt < BL * NG:
                                nb, ng = divmod(nxt, NG)
                                pending[(nb, ng)] = get_slab(nb, ng)

    nc.compile()
    return nc


def run(inputs, profile=False):
    import ml_dtypes
    from concourse.bass_utils import run_bass_kernel_spmd

    if "nc" not in _cache:
        _cache["nc"] = build()
    nc = _cache["nc"]

    x = np.ascontiguousarray(np.asarray(inputs["x"], dtype=np.float32))
    y = np.ascontiguousarray(np.asarray(inputs["y"], dtype=np.float32))
    dense_w = np.asarray(inputs["dense_w"], dtype=np.float32)
    dense_b = np.asarray(inputs["dense_b"], dtype=np.float32)
    weight = np.asarray(inputs["weight"], dtype=np.float32)

    G = np.array([[1 / 2, 0, 0],
                  [-1 / 2, -1 / 2, -1 / 2],
                  [-1 / 6, 1 / 6, -1 / 6],
                  [1 / 6, 1 / 3, 2 / 3],
                  [0, 0, 1]], np.float32)
    wm = weight * np.float32(W_MUL_CONV)
    wh = np.einsum('qk,oikw->iqwo', G, wm)                  # [i, 5, 3, o]
    whT = np.ascontiguousarray(
        wh.reshape(CIN, 15, OC, 128).transpose(2, 0, 1, 3)
        .astype(ml_dtypes.bfloat16))                        # [ot, i, tap, o]
    wsq_h = np.ascontiguousarray(
        np.sum(wm.astype(np.float64) ** 2, axis=(2, 3)).T
        .astype(ml_dtypes.bfloat16))
    dwt = np.ascontiguousarray(
        dense_w.T.reshape(4, 128, CIN).transpose(1, 0, 2)
        .astype(ml_dtypes.bfloat16))                        # [p, l-chunk, c]
    db = np.ascontiguousarray(
        dense_b.reshape(4, 128, 1).transpose(1, 0, 2))      # [p, c-chunk, 1]

    in_maps = []
    for c in range(N_CORES):
        sl = slice(c * BL, (c + 1) * BL)
        in_maps.append({
            "x": x[sl],
            "whT": whT,
            "wsq": wsq_h,
            "dwt": dwt,
            "yt": np.ascontiguousarray(y[sl].T.reshape(4, 128, BL).transpose(1, 0, 2).astype(ml_dtypes.bfloat16)),
            "db": db,
        })

    if profile:
        _ensure_ntff_hook()
    res = run_bass_kernel_spmd(
        nc, in_maps, core_ids=list(range(N_CORES)), trace=profile)
    out = np.concatenate([r["out"] for r in res.results], axis=0)
    return out, res.exec_time_ns


def kernel(**inputs) -> np.ndarray:
    out, _ = run(inputs)
    return out
